# revision 3
# baseline (speedup 1.0000x reference)
"""Bilinear resampling kernel for Trainium2 (8 NeuronCores, SPMD).

reference semantics:
    u = target_uv[:, 0] / downscale ; v = target_uv[:, 1] / downscale
    out[c, i] = bilinear sample of feature_map[c] at (v[i], u[i])   -> [C, N]

Strategy (v2 — host-side transpose, bf16, PE blend)
---------------------------------------------------
Host: sort points by v, split into 8 equal per-core chunks, then split each
core's chunk into B bands (span <= 26 map rows so pixel indices fit int16).
For each band ship a PAIR-INTERLEAVED pixel-major bf16 slice pm:
    pm[r*1248 + u] = [fm[:, base+r, u], fm[:, base+r+1, u]]   (256 bf16)
so ONE gather window of 512 bf16 (= 2 consecutive pixel-pairs, 1KB) holds
all four bilinear corners of a point.  Weights (f32) and gather indices
(int16, wrapped [16, n/16] + replicated x8) are precomputed on host.

Device (same program on all 8 cores):
  per chunk of CT tiles (128 points each): one dma_gather pulls 1KB per
  point -> G[p, j, 512] (partition = point).  Blend + transpose happen ON
  THE PE: for each corner k, build D_k = diag(w_k) with one tensor_scalar
  (identity x per-partition weight, 4x DVE mode), then accumulate
      psum[c, p] += G_k[p, c]^T @ D_k        (4 matmuls, one PSUM group)
  Four tiles share one PSUM bank; one ScalarE copy downcasts to bf16 into
  the output buffer, DMA to out.  Output returned bf16, host converts.

Engine budget per core (cost model): DMA ~94us (gather 25.6MB + out 6.4MB),
DVE ~74us, PE ~45us, Act ~30us, Pool ~35us  ->  DMA-bound.
"""

import numpy as np
import ml_dtypes

import concourse.bacc as bacc
import concourse.bass as bass
import concourse.mybir as mybir
import concourse.tile as tile
from concourse.bass_utils import run_bass_kernel_spmd
from concourse.masks import make_identity

C = 128
P = 128
W2 = 1248          # row pitch in pixels
H_FULL = 376
N_CORES = 8
F32 = mybir.dt.float32
BF16 = mybir.dt.bfloat16
I16 = mybir.dt.int16
BFNP = ml_dtypes.bfloat16
SPAN_MAX = 26      # (span-1)*1248 + 1246 <= 32767 (int16 gather idx)


def build_program(B, SPANS, Tb, num_devices=N_CORES, CT=8,
                  gbufs=6, dbufs=12, obufs=6, pbufs=8):
    """SPMD Bass program. SPANS = per-band pm row counts (shared across cores)."""
    if isinstance(SPANS, int):
        SPANS = (SPANS,) * B
    SPANS = tuple(SPANS)
    assert len(SPANS) == B
    NPIX = [s * W2 for s in SPANS]          # pixel-pairs per band
    FOFF = [0]
    for n in NPIX:
        FOFF.append(FOFF[-1] + n)
    Tc = B * Tb

    nc = bacc.Bacc("TRN2", target_bir_lowering=False, debug=False,
                   num_devices=num_devices, num_swdge_queues=2)

    pm = nc.dram_tensor("pm", [FOFF[-1], 256], BF16, kind="ExternalInput")
    wts = nc.dram_tensor("wts", [P, 4 * Tc], F32, kind="ExternalInput")
    idxs = nc.dram_tensor("idxs", [P, Tc * 8], I16, kind="ExternalInput")
    out = nc.dram_tensor("out", [P, Tc * P], BF16, kind="ExternalOutput")

    with tile.TileContext(nc) as tc:
        with (
            tc.tile_pool(name="const", bufs=1) as cpool,
            tc.tile_pool(name="gather", bufs=gbufs) as gpool,
            tc.tile_pool(name="diag", bufs=dbufs) as dpool,
            tc.tile_pool(name="obuf", bufs=obufs) as opool,
            tc.tile_pool(name="psum", bufs=pbufs, space="PSUM") as ppool,
        ):
            ident = cpool.tile([P, P], BF16, tag="ident")
            make_identity(nc, ident[:])
            wt = cpool.tile([P, 4 * Tc], F32, tag="wt")
            it = cpool.tile([P, Tc * 8], I16, tag="it")
            # split preamble loads per band so the first gather starts early
            for b in range(B):
                nc.sync.dma_start(out=it[:, b * Tb * 8:(b + 1) * Tb * 8],
                                  in_=idxs[:, b * Tb * 8:(b + 1) * Tb * 8])
                nc.sync.dma_start(out=wt[:, 4 * b * Tb:4 * (b + 1) * Tb],
                                  in_=wts[:, 4 * b * Tb:4 * (b + 1) * Tb])

            # chunk sizes: small first chunk (quick pipeline fill), small
            # final chunks (short drain), CT in the middle
            def chunk_sizes(nt, first, last):
                szs = []
                rem = nt
                if first and rem > CT:
                    szs.append(2)
                    rem -= 2
                tail = [2, 2] if (last and rem > 6) else []
                body = rem - sum(tail)
                while body > 0:
                    c = min(CT, body)
                    szs.append(c)
                    body -= c
                szs.extend(tail)
                return szs

            qn = 0
            for b in range(B):
                src_ap = bass.AP(pm[:].tensor, FOFF[b] * 256,
                                 [[256, NPIX[b] - 1], [1, 512]])
                t0 = 0
                for ct in chunk_sizes(Tb, b == 0, b == B - 1):
                    tg = b * Tb + t0        # global tile id of chunk start
                    t0 += ct
                    G = gpool.tile([P, CT * 512], BF16, tag="G")
                    nc.gpsimd.dma_gather(
                        out_ap=G[:, :ct * 512].rearrange(
                            "p (n e) -> p n e", e=512),
                        in_ap=src_ap,
                        idxs_ap=it[:, tg * 8:(tg + ct) * 8],
                        num_idxs=ct * P,
                        num_idxs_reg=ct * P,
                        elem_size=512,
                        elem_step=256,
                        queue_num=qn,
                    )
                    qn ^= 1
                    ob = opool.tile([P, CT * P], BF16, tag="ob")
                    for g0 in range(0, ct, 4):
                        gw = min(4, ct - g0)
                        ps = ppool.tile([P, 4 * P], F32, tag="ps")
                        for jj in range(gw):
                            j = g0 + jj
                            t = tg + j
                            D = dpool.tile([P, 512], BF16, tag="D")
                            for k in range(4):
                                nc.vector.tensor_scalar_mul(
                                    D[:, k * P:(k + 1) * P], ident[:],
                                    wt[:, 4 * t + k:4 * t + k + 1])
                            for k in range(4):
                                nc.tensor.matmul(
                                    ps[:, jj * P:(jj + 1) * P],
                                    G[:, j * 512 + k * P:j * 512 + (k + 1) * P],
                                    D[:, k * P:(k + 1) * P],
                                    start=(k == 0), stop=(k == 3))
                        nc.scalar.copy(ob[:, g0 * P:(g0 + gw) * P],
                                       ps[:, :gw * P])
                    nc.sync.dma_start(
                        out=out[:, tg * P:(tg + ct) * P],
                        in_=ob[:, :ct * P])

    nc.compile()
    return nc


_PROGRAM_CACHE = {}


def _get_program(B, SPANS, Tb):
    key = (B, tuple(SPANS), Tb)
    if key not in _PROGRAM_CACHE:
        _PROGRAM_CACHE[key] = build_program(B, SPANS, Tb)
    return _PROGRAM_CACHE[key]


def kernel(feature_map, target_uv, downscale):
    fm = np.asarray(feature_map, dtype=np.float32)
    uv = np.asarray(target_uv, dtype=np.float32)
    ds = np.float32(np.asarray(downscale).item() if hasattr(downscale, "item")
                    else downscale)
    Cc, H, W = fm.shape
    N = uv.shape[0]
    assert Cc == C and W == W2

    u = (uv[:, 0] / ds).astype(np.float32)
    v = (uv[:, 1] / ds).astype(np.float32)
    ulo = u.astype(np.int32)
    vlo = v.astype(np.int32)
    du = u - ulo.astype(np.float32)
    dv = v - vlo.astype(np.float32)
    # corner order k: 0=(lo,u) 1=(hi,u) 2=(lo,u+1) 3=(hi,u+1)
    w4 = np.stack([(1 - dv) * (1 - du), dv * (1 - du),
                   (1 - dv) * du, dv * du], axis=1).astype(np.float32)

    order = np.argsort(v, kind="stable")
    core_bounds = [(N * k) // N_CORES for k in range(N_CORES + 1)]
    max_core_n = max(core_bounds[k + 1] - core_bounds[k]
                     for k in range(N_CORES))

    # choose band count B so that every band spans <= SPAN_MAX map rows
    B = 2
    while True:
        Tb = max(int(np.ceil(np.ceil(max_core_n / B) / P)), 1)
        NB = Tb * P
        bases = np.zeros((N_CORES, B), dtype=np.int64)
        spans = []
        band_pts = {}
        band_nreal = {}
        ok = True
        for k in range(N_CORES):
            ids = order[core_bounds[k]:core_bounds[k + 1]]
            nb_bounds = [(len(ids) * b) // B for b in range(B + 1)]
            for b in range(B):
                bids = ids[nb_bounds[b]:nb_bounds[b + 1]]
                if len(bids) == 0:
                    bids = ids[:1] if len(ids) else np.array([0], np.int64)
                vb = vlo[bids]
                base = int(vb.min())
                span = int(vb.max()) + 1 - base
                if span > SPAN_MAX:
                    ok = False
                    break
                bases[k, b] = base
                spans.append(span)
                band_nreal[(k, b)] = nb_bounds[b + 1] - nb_bounds[b]
                pad = NB - len(bids)
                band_pts[(k, b)] = np.concatenate(
                    [bids, np.repeat(bids[:1], pad)]) if pad else bids
            if not ok:
                break
        if ok:
            break
        B += 1

    spans2 = np.array(spans, dtype=np.int64).reshape(N_CORES, B)
    SPANS = tuple(int(spans2[:, b].max()) for b in range(B))
    NPIX = [s * W2 for s in SPANS]
    FOFF = [0]
    for n in NPIX:
        FOFF.append(FOFF[-1] + n)
    # clamp bases so base + SPANS[b] <= H - 1 (pm pair r uses rows r, r+1)
    for k in range(N_CORES):
        for b in range(B):
            bases[k, b] = min(bases[k, b], H - 1 - SPANS[b])
    Tc = B * Tb

    fmT16 = fm.transpose(1, 2, 0).astype(BFNP)      # [H, W, C] bf16

    in_maps = []
    for k in range(N_CORES):
        pm_k = np.empty((FOFF[-1], 256), dtype=BFNP)
        wts_k = np.empty((P, 4 * Tc), dtype=np.float32)
        idx_k = np.empty((16, Tc * 8), dtype=np.int16)
        for b in range(B):
            base = int(bases[k, b])
            span = SPANS[b]
            blk = np.stack([fmT16[base:base + span],
                            fmT16[base + 1:base + 1 + span]], axis=2)
            pm_k[FOFF[b]:FOFF[b + 1]] = blk.reshape(span * W2, 256)
            pts = band_pts[(k, b)]
            # weights: [p, 4*t] layout (tile-major cols, 4 per tile)
            wb = w4[pts].reshape(Tb, P, 4).transpose(1, 0, 2).reshape(P, Tb * 4)
            wts_k[:, 4 * b * Tb:4 * (b + 1) * Tb] = wb
            # gather indices: element (t*128+p) at [p%16, t*8 + p//16]
            pix = ((vlo[pts] - base).astype(np.int64) * W2
                   + ulo[pts]).astype(np.int16)
            ib = pix.reshape(Tb, 8, 16).transpose(2, 0, 1).reshape(16, Tb * 8)
            idx_k[:, b * Tb * 8:(b + 1) * Tb * 8] = ib
        in_maps.append({"pm": pm_k, "wts": wts_k,
                        "idxs": np.tile(idx_k, (8, 1))})

    nc = _get_program(B, SPANS, Tb)
    res = run_bass_kernel_spmd(nc, in_maps, list(range(N_CORES)))

    out_full = np.empty((C, N), dtype=np.float32)
    for k in range(N_CORES):
        ok_arr = np.asarray(res.results[k]["out"]).astype(np.float32)
        for b in range(B):
            pts = band_pts[(k, b)]
            nreal = band_nreal[(k, b)]
            out_full[:, pts[:nreal]] = ok_arr[:, b * Tb * P:b * Tb * P + nreal]
    return out_full


# revision 4
# speedup vs baseline: 1.0042x; 1.0042x over previous
"""Bilinear resampling kernel for Trainium2 (8 NeuronCores, SPMD).

reference semantics:
    u = target_uv[:, 0] / downscale ; v = target_uv[:, 1] / downscale
    out[c, i] = bilinear sample of feature_map[c] at (v[i], u[i])   -> [C, N]

Strategy (v2 — host-side transpose, bf16, PE blend)
---------------------------------------------------
Host: sort points by v, split into 8 equal per-core chunks, then split each
core's chunk into B bands (span <= 26 map rows so pixel indices fit int16).
For each band ship a PAIR-INTERLEAVED pixel-major bf16 slice pm:
    pm[r*1248 + u] = [fm[:, base+r, u], fm[:, base+r+1, u]]   (256 bf16)
so ONE gather window of 512 bf16 (= 2 consecutive pixel-pairs, 1KB) holds
all four bilinear corners of a point.  Weights (f32) and gather indices
(int16, wrapped [16, n/16] + replicated x8) are precomputed on host.

Device (same program on all 8 cores):
  per chunk of CT tiles (128 points each): one dma_gather pulls 1KB per
  point -> G[p, j, 512] (partition = point).  Blend + transpose happen ON
  THE PE: for each corner k, build D_k = diag(w_k) with one tensor_scalar
  (identity x per-partition weight, 4x DVE mode), then accumulate
      psum[c, p] += G_k[p, c]^T @ D_k        (4 matmuls, one PSUM group)
  Four tiles share one PSUM bank; one ScalarE copy downcasts to bf16 into
  the output buffer, DMA to out.  Output returned bf16, host converts.

Engine budget per core (cost model): DMA ~94us (gather 25.6MB + out 6.4MB),
DVE ~74us, PE ~45us, Act ~30us, Pool ~35us  ->  DMA-bound.
"""

import numpy as np
import ml_dtypes

import concourse.bacc as bacc
import concourse.bass as bass
import concourse.mybir as mybir
import concourse.tile as tile
from concourse.bass_utils import run_bass_kernel_spmd
from concourse.masks import make_identity

C = 128
P = 128
W2 = 1248          # row pitch in pixels
H_FULL = 376
N_CORES = 8
F32 = mybir.dt.float32
BF16 = mybir.dt.bfloat16
I16 = mybir.dt.int16
BFNP = ml_dtypes.bfloat16
SPAN_MAX = 26      # (span-1)*1248 + 1246 <= 32767 (int16 gather idx)


def build_program(B, SPANS, Tb, num_devices=N_CORES, CT=8,
                  gbufs=6, dbufs=12, obufs=6, pbufs=8):
    """SPMD Bass program. SPANS = per-band pm row counts (shared across cores)."""
    if isinstance(SPANS, int):
        SPANS = (SPANS,) * B
    SPANS = tuple(SPANS)
    assert len(SPANS) == B
    NPIX = [s * W2 for s in SPANS]          # pixel-pairs per band
    FOFF = [0]
    for n in NPIX:
        FOFF.append(FOFF[-1] + n)
    Tc = B * Tb

    nc = bacc.Bacc("TRN2", target_bir_lowering=False, debug=False,
                   num_devices=num_devices, num_swdge_queues=2)

    pm = nc.dram_tensor("pm", [FOFF[-1], 256], BF16, kind="ExternalInput")
    wts = nc.dram_tensor("wts", [P, 4 * Tc], F32, kind="ExternalInput")
    idxs = nc.dram_tensor("idxs", [P, Tc * 8], I16, kind="ExternalInput")
    out = nc.dram_tensor("out", [P, Tc * P], BF16, kind="ExternalOutput")

    with tile.TileContext(nc) as tc:
        with (
            tc.tile_pool(name="const", bufs=1) as cpool,
            tc.tile_pool(name="gather", bufs=gbufs) as gpool,
            tc.tile_pool(name="diag", bufs=dbufs) as dpool,
            tc.tile_pool(name="obuf", bufs=obufs) as opool,
            tc.tile_pool(name="psum", bufs=pbufs, space="PSUM") as ppool,
        ):
            ident = cpool.tile([P, P], BF16, tag="ident")
            make_identity(nc, ident[:])
            wt = cpool.tile([P, 4 * Tc], F32, tag="wt")
            it = cpool.tile([P, Tc * 8], I16, tag="it")
            # split preamble loads so the first gather starts ASAP
            nc.sync.dma_start(out=it[:, 0:16], in_=idxs[:, 0:16])
            for b in range(B):
                lo = b * Tb * 8 + (16 if b == 0 else 0)
                nc.sync.dma_start(out=it[:, lo:(b + 1) * Tb * 8],
                                  in_=idxs[:, lo:(b + 1) * Tb * 8])
                nc.sync.dma_start(out=wt[:, 4 * b * Tb:4 * (b + 1) * Tb],
                                  in_=wts[:, 4 * b * Tb:4 * (b + 1) * Tb])

            # chunk sizes: small first chunk (quick pipeline fill), small
            # final chunks (short drain), CT in the middle
            def chunk_sizes(nt, first, last):
                szs = []
                rem = nt
                if first and rem > CT:
                    szs.append(2)
                    rem -= 2
                tail = [2, 1] if (last and rem > 6) else []
                body = rem - sum(tail)
                while body > 0:
                    c = min(CT, body)
                    szs.append(c)
                    body -= c
                szs.extend(tail)
                return szs

            qn = 0
            for b in range(B):
                src_ap = bass.AP(pm[:].tensor, FOFF[b] * 256,
                                 [[256, NPIX[b] - 1], [1, 512]])
                t0 = 0
                for ct in chunk_sizes(Tb, b == 0, b == B - 1):
                    tg = b * Tb + t0        # global tile id of chunk start
                    t0 += ct
                    G = gpool.tile([P, CT * 512], BF16, tag="G")
                    nc.gpsimd.dma_gather(
                        out_ap=G[:, :ct * 512].rearrange(
                            "p (n e) -> p n e", e=512),
                        in_ap=src_ap,
                        idxs_ap=it[:, tg * 8:(tg + ct) * 8],
                        num_idxs=ct * P,
                        num_idxs_reg=ct * P,
                        elem_size=512,
                        elem_step=256,
                        queue_num=qn,
                    )
                    qn ^= 1
                    ob = opool.tile([P, CT * P], BF16, tag="ob")
                    for g0 in range(0, ct, 4):
                        gw = min(4, ct - g0)
                        ps = ppool.tile([P, 4 * P], F32, tag="ps")
                        for jj in range(gw):
                            j = g0 + jj
                            t = tg + j
                            D = dpool.tile([P, 512], BF16, tag="D")
                            for k in range(4):
                                nc.vector.tensor_scalar_mul(
                                    D[:, k * P:(k + 1) * P], ident[:],
                                    wt[:, 4 * t + k:4 * t + k + 1])
                            for k in range(4):
                                nc.tensor.matmul(
                                    ps[:, jj * P:(jj + 1) * P],
                                    G[:, j * 512 + k * P:j * 512 + (k + 1) * P],
                                    D[:, k * P:(k + 1) * P],
                                    start=(k == 0), stop=(k == 3))
                        nc.scalar.copy(ob[:, g0 * P:(g0 + gw) * P],
                                       ps[:, :gw * P])
                    nc.sync.dma_start(
                        out=out[:, tg * P:(tg + ct) * P],
                        in_=ob[:, :ct * P])

    nc.compile()
    return nc


_PROGRAM_CACHE = {}


def _get_program(B, SPANS, Tb):
    key = (B, tuple(SPANS), Tb)
    if key not in _PROGRAM_CACHE:
        _PROGRAM_CACHE[key] = build_program(B, SPANS, Tb)
    return _PROGRAM_CACHE[key]


def kernel(feature_map, target_uv, downscale):
    fm = np.asarray(feature_map, dtype=np.float32)
    uv = np.asarray(target_uv, dtype=np.float32)
    ds = np.float32(np.asarray(downscale).item() if hasattr(downscale, "item")
                    else downscale)
    Cc, H, W = fm.shape
    N = uv.shape[0]
    assert Cc == C and W == W2

    u = (uv[:, 0] / ds).astype(np.float32)
    v = (uv[:, 1] / ds).astype(np.float32)
    ulo = u.astype(np.int32)
    vlo = v.astype(np.int32)
    du = u - ulo.astype(np.float32)
    dv = v - vlo.astype(np.float32)
    # corner order k: 0=(lo,u) 1=(hi,u) 2=(lo,u+1) 3=(hi,u+1)
    w4 = np.stack([(1 - dv) * (1 - du), dv * (1 - du),
                   (1 - dv) * du, dv * du], axis=1).astype(np.float32)

    order = np.argsort(v, kind="stable")
    core_bounds = [(N * k) // N_CORES for k in range(N_CORES + 1)]
    max_core_n = max(core_bounds[k + 1] - core_bounds[k]
                     for k in range(N_CORES))

    # choose band count B so that every band spans <= SPAN_MAX map rows
    B = 2
    while True:
        Tb = max(int(np.ceil(np.ceil(max_core_n / B) / P)), 1)
        NB = Tb * P
        bases = np.zeros((N_CORES, B), dtype=np.int64)
        spans = []
        band_pts = {}
        band_nreal = {}
        ok = True
        for k in range(N_CORES):
            ids = order[core_bounds[k]:core_bounds[k + 1]]
            nb_bounds = [(len(ids) * b) // B for b in range(B + 1)]
            for b in range(B):
                bids = ids[nb_bounds[b]:nb_bounds[b + 1]]
                if len(bids) == 0:
                    bids = ids[:1] if len(ids) else np.array([0], np.int64)
                vb = vlo[bids]
                base = int(vb.min())
                span = int(vb.max()) + 1 - base
                if span > SPAN_MAX:
                    ok = False
                    break
                bases[k, b] = base
                spans.append(span)
                band_nreal[(k, b)] = nb_bounds[b + 1] - nb_bounds[b]
                pad = NB - len(bids)
                band_pts[(k, b)] = np.concatenate(
                    [bids, np.repeat(bids[:1], pad)]) if pad else bids
            if not ok:
                break
        if ok:
            break
        B += 1

    spans2 = np.array(spans, dtype=np.int64).reshape(N_CORES, B)
    SPANS = tuple(int(spans2[:, b].max()) for b in range(B))
    NPIX = [s * W2 for s in SPANS]
    FOFF = [0]
    for n in NPIX:
        FOFF.append(FOFF[-1] + n)
    # clamp bases so base + SPANS[b] <= H - 1 (pm pair r uses rows r, r+1)
    for k in range(N_CORES):
        for b in range(B):
            bases[k, b] = min(bases[k, b], H - 1 - SPANS[b])
    Tc = B * Tb

    fmT16 = fm.transpose(1, 2, 0).astype(BFNP)      # [H, W, C] bf16

    in_maps = []
    for k in range(N_CORES):
        pm_k = np.empty((FOFF[-1], 256), dtype=BFNP)
        wts_k = np.empty((P, 4 * Tc), dtype=np.float32)
        idx_k = np.empty((16, Tc * 8), dtype=np.int16)
        for b in range(B):
            base = int(bases[k, b])
            span = SPANS[b]
            blk = np.stack([fmT16[base:base + span],
                            fmT16[base + 1:base + 1 + span]], axis=2)
            pm_k[FOFF[b]:FOFF[b + 1]] = blk.reshape(span * W2, 256)
            pts = band_pts[(k, b)]
            # weights: [p, 4*t] layout (tile-major cols, 4 per tile)
            wb = w4[pts].reshape(Tb, P, 4).transpose(1, 0, 2).reshape(P, Tb * 4)
            wts_k[:, 4 * b * Tb:4 * (b + 1) * Tb] = wb
            # gather indices: element (t*128+p) at [p%16, t*8 + p//16]
            pix = ((vlo[pts] - base).astype(np.int64) * W2
                   + ulo[pts]).astype(np.int16)
            ib = pix.reshape(Tb, 8, 16).transpose(2, 0, 1).reshape(16, Tb * 8)
            idx_k[:, b * Tb * 8:(b + 1) * Tb * 8] = ib
        in_maps.append({"pm": pm_k, "wts": wts_k,
                        "idxs": np.tile(idx_k, (8, 1))})

    nc = _get_program(B, SPANS, Tb)
    res = run_bass_kernel_spmd(nc, in_maps, list(range(N_CORES)))

    out_full = np.empty((C, N), dtype=np.float32)
    for k in range(N_CORES):
        ok_arr = np.asarray(res.results[k]["out"]).astype(np.float32)
        for b in range(B):
            pts = band_pts[(k, b)]
            nreal = band_nreal[(k, b)]
            out_full[:, pts[:nreal]] = ok_arr[:, b * Tb * P:b * Tb * P + nreal]
    return out_full


# revision 5
# speedup vs baseline: 1.0104x; 1.0062x over previous
"""Bilinear resampling kernel for Trainium2 (8 NeuronCores, SPMD).

reference semantics:
    u = target_uv[:, 0] / downscale ; v = target_uv[:, 1] / downscale
    out[c, i] = bilinear sample of feature_map[c] at (v[i], u[i])   -> [C, N]

Strategy (v2 — host-side transpose, bf16, PE blend)
---------------------------------------------------
Host: sort points by v, split into 8 equal per-core chunks, then split each
core's chunk into B bands (span <= 26 map rows so pixel indices fit int16).
For each band ship a PAIR-INTERLEAVED pixel-major bf16 slice pm:
    pm[r*1248 + u] = [fm[:, base+r, u], fm[:, base+r+1, u]]   (256 bf16)
so ONE gather window of 512 bf16 (= 2 consecutive pixel-pairs, 1KB) holds
all four bilinear corners of a point.  Weights (f32) and gather indices
(int16, wrapped [16, n/16] + replicated x8) are precomputed on host.

Device (same program on all 8 cores):
  per chunk of CT tiles (128 points each): one dma_gather pulls 1KB per
  point -> G[p, j, 512] (partition = point).  Blend + transpose happen ON
  THE PE: for each corner k, build D_k = diag(w_k) with one tensor_scalar
  (identity x per-partition weight, 4x DVE mode), then accumulate
      psum[c, p] += G_k[p, c]^T @ D_k        (4 matmuls, one PSUM group)
  Four tiles share one PSUM bank; one ScalarE copy downcasts to bf16 into
  the output buffer, DMA to out.  Output returned bf16, host converts.

Engine budget per core (cost model): DMA ~94us (gather 25.6MB + out 6.4MB),
DVE ~74us, PE ~45us, Act ~30us, Pool ~35us  ->  DMA-bound.
"""

import numpy as np
import ml_dtypes

import concourse.bacc as bacc
import concourse.bass as bass
import concourse.mybir as mybir
import concourse.tile as tile
from concourse.bass_utils import run_bass_kernel_spmd
from concourse.masks import make_identity

C = 128
P = 128
W2 = 1248          # row pitch in pixels
H_FULL = 376
N_CORES = 8
F32 = mybir.dt.float32
BF16 = mybir.dt.bfloat16
I16 = mybir.dt.int16
BFNP = ml_dtypes.bfloat16
SPAN_MAX = 26      # (span-1)*1248 + 1246 <= 32767 (int16 gather idx)


def build_program(B, SPANS, Tb, num_devices=N_CORES, CT=8,
                  gbufs=6, dbufs=12, obufs=6, pbufs=8):
    """SPMD Bass program. SPANS = per-band pm row counts (shared across cores)."""
    if isinstance(SPANS, int):
        SPANS = (SPANS,) * B
    SPANS = tuple(SPANS)
    assert len(SPANS) == B
    PFXT = 2 if Tb > CT else 0              # prefix tiles w/ iota-gen indices
    PFX = PFXT * 256                        # prefix pixel-pair rows
    NPIX = [s * W2 for s in SPANS]          # pixel-pairs per band
    FOFF = [PFX]
    for n in NPIX:
        FOFF.append(FOFF[-1] + n)
    Tc = B * Tb

    nc = bacc.Bacc("TRN2", target_bir_lowering=False, debug=False,
                   num_devices=num_devices, num_swdge_queues=2)

    pm = nc.dram_tensor("pm", [FOFF[-1], 256], BF16, kind="ExternalInput")
    wts = nc.dram_tensor("wts", [P, 4 * Tc], F32, kind="ExternalInput")
    idxs = nc.dram_tensor("idxs", [P, Tc * 8], I16, kind="ExternalInput")
    out = nc.dram_tensor("out", [P, Tc * P], BF16, kind="ExternalOutput")

    with tile.TileContext(nc) as tc:
        with (
            tc.tile_pool(name="const", bufs=1) as cpool,
            tc.tile_pool(name="gather", bufs=gbufs) as gpool,
            tc.tile_pool(name="diag", bufs=dbufs) as dpool,
            tc.tile_pool(name="obuf", bufs=obufs) as opool,
            tc.tile_pool(name="psum", bufs=pbufs, space="PSUM") as ppool,
        ):
            ident = cpool.tile([P, P], BF16, tag="ident")
            wt = cpool.tile([P, 4 * Tc], F32, tag="wt")
            it = cpool.tile([P, Tc * 8], I16, tag="it")
            G0 = None
            if PFXT:
                # iota-generated idx for the prefix chunk: no DMA dependency.
                # replicated wrapped layout: it0[p, c] = 32*c + 2*(p % 16)
                it0 = cpool.tile([P, PFXT * 8], I16, tag="it0")
                itp = cpool.tile([P, PFXT * 8], I16, tag="itp")
                nc.gpsimd.iota(itp[:], pattern=[[0, PFXT * 8]],
                               base=0, channel_multiplier=1)
                nc.vector.tensor_scalar(itp[:], itp[:], 15, None,
                                        op0=mybir.AluOpType.bitwise_and)
                nc.gpsimd.iota(it0[:], pattern=[[32, PFXT * 8]],
                               base=0, channel_multiplier=0)
                nc.vector.scalar_tensor_tensor(
                    it0[:], itp[:], 2, it0[:],
                    op0=mybir.AluOpType.mult, op1=mybir.AluOpType.add)
                # issue the prefix gather before anything else queues on Pool
                G0 = gpool.tile([P, CT * 512], BF16, tag="G")
                nc.gpsimd.dma_gather(
                    out_ap=G0[:, :PFXT * 512].rearrange(
                        "p (n e) -> p n e", e=512),
                    in_ap=bass.AP(pm[:].tensor, 0,
                                  [[256, PFX - 1], [1, 512]]),
                    idxs_ap=it0[:, :PFXT * 8],
                    num_idxs=PFXT * P,
                    num_idxs_reg=PFXT * P,
                    elem_size=512,
                    elem_step=256,
                    queue_num=0,
                )
            make_identity(nc, ident[:])
            # split preamble loads so later gathers' idx arrive in time
            for b in range(B):
                lo = b * Tb * 8
                nc.sync.dma_start(out=it[:, lo:(b + 1) * Tb * 8],
                                  in_=idxs[:, lo:(b + 1) * Tb * 8])
                nc.sync.dma_start(out=wt[:, 4 * b * Tb:4 * (b + 1) * Tb],
                                  in_=wts[:, 4 * b * Tb:4 * (b + 1) * Tb])

            # chunk sizes: small first chunk (quick pipeline fill), small
            # final chunks (short drain), CT in the middle
            def chunk_sizes(nt, first, last):
                szs = []
                rem = nt
                if first and rem > CT:
                    szs.append(2)
                    rem -= 2
                tail = [2, 1] if (last and rem > 6) else []
                body = rem - sum(tail)
                while body > 0:
                    c = min(CT, body)
                    szs.append(c)
                    body -= c
                szs.extend(tail)
                return szs

            # chunk descriptors: (band, first tile, size)
            chunks = []
            for b in range(B):
                t0 = 0
                for ct in chunk_sizes(Tb, b == 0, b == B - 1):
                    chunks.append((b, b * Tb + t0, ct))
                    t0 += ct
            qn = 0
            for i, (b, tg, ct) in enumerate(chunks):
                src_ap = bass.AP(pm[:].tensor, FOFF[b] * 256,
                                 [[256, NPIX[b] - 1], [1, 512]])
                is_pfx = PFXT and i == 0
                if is_pfx:
                    G = G0          # gathered up-front via iota idx
                else:
                    G = gpool.tile([P, CT * 512], BF16, tag="G")
                    nc.gpsimd.dma_gather(
                        out_ap=G[:, :ct * 512].rearrange(
                            "p (n e) -> p n e", e=512),
                        in_ap=src_ap,
                        idxs_ap=it[:, tg * 8:(tg + ct) * 8],
                        num_idxs=ct * P,
                        num_idxs_reg=ct * P,
                        elem_size=512,
                        elem_step=256,
                        queue_num=qn,
                    )
                qn ^= 1
                ob = opool.tile([P, CT * P], BF16, tag="ob")
                for g0 in range(0, ct, 4):
                    gw = min(4, ct - g0)
                    ps = ppool.tile([P, 4 * P], F32, tag="ps")
                    for jj in range(gw):
                        j = g0 + jj
                        t = tg + j
                        D = dpool.tile([P, 512], BF16, tag="D")
                        for k in range(4):
                            nc.vector.tensor_scalar_mul(
                                D[:, k * P:(k + 1) * P], ident[:],
                                wt[:, 4 * t + k:4 * t + k + 1])
                        for k in range(4):
                            nc.tensor.matmul(
                                ps[:, jj * P:(jj + 1) * P],
                                G[:, j * 512 + k * P:j * 512 + (k + 1) * P],
                                D[:, k * P:(k + 1) * P],
                                start=(k == 0), stop=(k == 3))
                    nc.scalar.copy(ob[:, g0 * P:(g0 + gw) * P],
                                   ps[:, :gw * P])
                nc.sync.dma_start(
                    out=out[:, tg * P:(tg + ct) * P],
                    in_=ob[:, :ct * P])

    nc.compile()
    return nc


_PROGRAM_CACHE = {}


def _get_program(B, SPANS, Tb):
    key = (B, tuple(SPANS), Tb)
    if key not in _PROGRAM_CACHE:
        _PROGRAM_CACHE[key] = build_program(B, SPANS, Tb)
    return _PROGRAM_CACHE[key]


def kernel(feature_map, target_uv, downscale):
    fm = np.asarray(feature_map, dtype=np.float32)
    uv = np.asarray(target_uv, dtype=np.float32)
    ds = np.float32(np.asarray(downscale).item() if hasattr(downscale, "item")
                    else downscale)
    Cc, H, W = fm.shape
    N = uv.shape[0]
    assert Cc == C and W == W2

    u = (uv[:, 0] / ds).astype(np.float32)
    v = (uv[:, 1] / ds).astype(np.float32)
    ulo = u.astype(np.int32)
    vlo = v.astype(np.int32)
    du = u - ulo.astype(np.float32)
    dv = v - vlo.astype(np.float32)
    # corner order k: 0=(lo,u) 1=(hi,u) 2=(lo,u+1) 3=(hi,u+1)
    w4 = np.stack([(1 - dv) * (1 - du), dv * (1 - du),
                   (1 - dv) * du, dv * du], axis=1).astype(np.float32)

    order = np.argsort(v, kind="stable")
    core_bounds = [(N * k) // N_CORES for k in range(N_CORES + 1)]
    max_core_n = max(core_bounds[k + 1] - core_bounds[k]
                     for k in range(N_CORES))

    # choose band count B so that every band spans <= SPAN_MAX map rows
    B = 2
    while True:
        Tb = max(int(np.ceil(np.ceil(max_core_n / B) / P)), 1)
        NB = Tb * P
        bases = np.zeros((N_CORES, B), dtype=np.int64)
        spans = []
        band_pts = {}
        band_nreal = {}
        ok = True
        for k in range(N_CORES):
            ids = order[core_bounds[k]:core_bounds[k + 1]]
            nb_bounds = [(len(ids) * b) // B for b in range(B + 1)]
            for b in range(B):
                bids = ids[nb_bounds[b]:nb_bounds[b + 1]]
                if len(bids) == 0:
                    bids = ids[:1] if len(ids) else np.array([0], np.int64)
                vb = vlo[bids]
                base = int(vb.min())
                span = int(vb.max()) + 1 - base
                if span > SPAN_MAX:
                    ok = False
                    break
                bases[k, b] = base
                spans.append(span)
                band_nreal[(k, b)] = nb_bounds[b + 1] - nb_bounds[b]
                pad = NB - len(bids)
                band_pts[(k, b)] = np.concatenate(
                    [bids, np.repeat(bids[:1], pad)]) if pad else bids
            if not ok:
                break
        if ok:
            break
        B += 1

    spans2 = np.array(spans, dtype=np.int64).reshape(N_CORES, B)
    SPANS = tuple(int(spans2[:, b].max()) for b in range(B))
    NPIX = [s * W2 for s in SPANS]
    CT = 8
    PFXT = 2 if Tb > CT else 0
    PFX = PFXT * 256
    FOFF = [PFX]
    for n in NPIX:
        FOFF.append(FOFF[-1] + n)
    # clamp bases so base + SPANS[b] <= H - 1 (pm pair r uses rows r, r+1)
    for k in range(N_CORES):
        for b in range(B):
            bases[k, b] = min(bases[k, b], H - 1 - SPANS[b])
    Tc = B * Tb

    fmT16 = fm.transpose(1, 2, 0).astype(BFNP)      # [H, W, C] bf16

    in_maps = []
    for k in range(N_CORES):
        pm_k = np.empty((FOFF[-1], 256), dtype=BFNP)
        wts_k = np.empty((P, 4 * Tc), dtype=np.float32)
        idx_k = np.empty((16, Tc * 8), dtype=np.int16)
        for b in range(B):
            base = int(bases[k, b])
            span = SPANS[b]
            blk = np.stack([fmT16[base:base + span],
                            fmT16[base + 1:base + 1 + span]], axis=2)
            bandflat = blk.reshape(span * W2, 256)
            pm_k[FOFF[b]:FOFF[b + 1]] = bandflat
            pts = band_pts[(k, b)]
            if b == 0 and PFXT:
                # duplicate first PFXT*128 points' windows into the prefix:
                # window i at pixel-pair rows [2i, 2i+1]
                p0 = pts[:PFXT * P]
                pix0 = ((vlo[p0] - base).astype(np.int64) * W2 + ulo[p0])
                rows = np.stack([pix0, pix0 + 1], axis=1).reshape(-1)
                pm_k[0:PFX] = bandflat[rows]
            # weights: [p, 4*t] layout (tile-major cols, 4 per tile)
            wb = w4[pts].reshape(Tb, P, 4).transpose(1, 0, 2).reshape(P, Tb * 4)
            wts_k[:, 4 * b * Tb:4 * (b + 1) * Tb] = wb
            # gather indices: element (t*128+p) at [p%16, t*8 + p//16]
            pix = ((vlo[pts] - base).astype(np.int64) * W2
                   + ulo[pts]).astype(np.int16)
            ib = pix.reshape(Tb, 8, 16).transpose(2, 0, 1).reshape(16, Tb * 8)
            idx_k[:, b * Tb * 8:(b + 1) * Tb * 8] = ib
        in_maps.append({"pm": pm_k, "wts": wts_k,
                        "idxs": np.tile(idx_k, (8, 1))})

    nc = _get_program(B, SPANS, Tb)
    res = run_bass_kernel_spmd(nc, in_maps, list(range(N_CORES)))

    out_full = np.empty((C, N), dtype=np.float32)
    for k in range(N_CORES):
        ok_arr = np.asarray(res.results[k]["out"]).astype(np.float32)
        for b in range(B):
            pts = band_pts[(k, b)]
            nreal = band_nreal[(k, b)]
            out_full[:, pts[:nreal]] = ok_arr[:, b * Tb * P:b * Tb * P + nreal]
    return out_full


# revision 10
# speedup vs baseline: 1.0140x; 1.0036x over previous
"""Bilinear resampling kernel for Trainium2 (8 NeuronCores, SPMD).

reference semantics:
    u = target_uv[:, 0] / downscale ; v = target_uv[:, 1] / downscale
    out[c, i] = bilinear sample of feature_map[c] at (v[i], u[i])   -> [C, N]

Strategy (v2 — host-side transpose, bf16, PE blend)
---------------------------------------------------
Host: sort points by v, split into 8 equal per-core chunks, then split each
core's chunk into B bands (span <= 26 map rows so pixel indices fit int16).
For each band ship a PAIR-INTERLEAVED pixel-major bf16 slice pm:
    pm[r*1248 + u] = [fm[:, base+r, u], fm[:, base+r+1, u]]   (256 bf16)
so ONE gather window of 512 bf16 (= 2 consecutive pixel-pairs, 1KB) holds
all four bilinear corners of a point.  Weights (f32) and gather indices
(int16, wrapped [16, n/16] + replicated x8) are precomputed on host.

Device (same program on all 8 cores):
  per chunk of CT tiles (128 points each): one dma_gather pulls 1KB per
  point -> G[p, j, 512] (partition = point).  Blend + transpose happen ON
  THE PE: for each corner k, build D_k = diag(w_k) with one tensor_scalar
  (identity x per-partition weight, 4x DVE mode), then accumulate
      psum[c, p] += G_k[p, c]^T @ D_k        (4 matmuls, one PSUM group)
  Four tiles share one PSUM bank; one ScalarE copy downcasts to bf16 into
  the output buffer, DMA to out.  Output returned bf16, host converts.

Engine budget per core (cost model): DMA ~94us (gather 25.6MB + out 6.4MB),
DVE ~74us, PE ~45us, Act ~30us, Pool ~35us  ->  DMA-bound.
"""

import numpy as np
import ml_dtypes

import concourse.bacc as bacc
import concourse.bass as bass
import concourse.mybir as mybir
import concourse.tile as tile
from concourse.bass_utils import run_bass_kernel_spmd
from concourse.masks import make_identity

C = 128
P = 128
W2 = 1248          # row pitch in pixels
H_FULL = 376
N_CORES = 8
F32 = mybir.dt.float32
BF16 = mybir.dt.bfloat16
I16 = mybir.dt.int16
BFNP = ml_dtypes.bfloat16
SPAN_MAX = 26      # (span-1)*1248 + 1246 <= 32767 (int16 gather idx)


def build_program(B, SPANS, Tb, num_devices=N_CORES, CT=8,
                  gbufs=6, dbufs=12, obufs=6, pbufs=8):
    """SPMD Bass program. SPANS = per-band pm row counts (shared across cores)."""
    if isinstance(SPANS, int):
        SPANS = (SPANS,) * B
    SPANS = tuple(SPANS)
    assert len(SPANS) == B
    PFXT = (2 + CT) if Tb > 2 * CT else 0   # prefix tiles w/ iota-gen indices
    PFX = PFXT * 256                        # prefix pixel-pair rows
    NPIX = [s * W2 for s in SPANS]          # pixel-pairs per band
    FOFF = [PFX]
    for n in NPIX:
        FOFF.append(FOFF[-1] + n)
    Tc = B * Tb

    nc = bacc.Bacc("TRN2", target_bir_lowering=False, debug=False,
                   num_devices=num_devices, num_swdge_queues=2)

    pm = nc.dram_tensor("pm", [FOFF[-1], 256], BF16, kind="ExternalInput")
    wts = nc.dram_tensor("wts", [P, 4 * Tc], F32, kind="ExternalInput")
    idxs = nc.dram_tensor("idxs", [P, Tc * 8], I16, kind="ExternalInput")
    out = nc.dram_tensor("out", [P, Tc * P], BF16, kind="ExternalOutput")

    with tile.TileContext(nc) as tc:
        with (
            tc.tile_pool(name="const", bufs=1) as cpool,
            tc.tile_pool(name="gather", bufs=gbufs) as gpool,
            tc.tile_pool(name="diag", bufs=dbufs) as dpool,
            tc.tile_pool(name="obuf", bufs=obufs) as opool,
            tc.tile_pool(name="psum", bufs=pbufs, space="PSUM") as ppool,
        ):
            ident = cpool.tile([P, P], BF16, tag="ident")
            wt = cpool.tile([P, 4 * Tc], F32, tag="wt")
            it = cpool.tile([P, Tc * 8], I16, tag="it")
            G0s = None
            if PFXT:
                # iota-generated idx for the prefix chunk: no DMA dependency.
                # replicated wrapped layout: it0[p, c] = 32*c + 2*(p % 16)
                it0 = cpool.tile([P, PFXT * 8], I16, tag="it0")
                itp = cpool.tile([P, PFXT * 8], I16, tag="itp")
                nc.gpsimd.iota(itp[:], pattern=[[0, PFXT * 8]],
                               base=0, channel_multiplier=1)
                nc.vector.tensor_scalar(itp[:], itp[:], 15, None,
                                        op0=mybir.AluOpType.bitwise_and)
                nc.gpsimd.iota(it0[:], pattern=[[32, PFXT * 8]],
                               base=0, channel_multiplier=0)
                nc.vector.scalar_tensor_tensor(
                    it0[:], itp[:], 2, it0[:],
                    op0=mybir.AluOpType.mult, op1=mybir.AluOpType.add)
                # issue the prefix gathers before anything else queues on
                # Pool; they cover the first chunks (sizes 2 then CT)
                src_pfx = bass.AP(pm[:].tensor, 0,
                                  [[256, PFX - 1], [1, 512]])
                G0s = []
                c0 = 0
                for pct in (2, CT):
                    Gp = gpool.tile([P, CT * 512], BF16, tag="G")
                    nc.gpsimd.dma_gather(
                        out_ap=Gp[:, :pct * 512].rearrange(
                            "p (n e) -> p n e", e=512),
                        in_ap=src_pfx,
                        idxs_ap=it0[:, c0 * 8:(c0 + pct) * 8],
                        num_idxs=pct * P,
                        num_idxs_reg=pct * P,
                        elem_size=512,
                        elem_step=256,
                        queue_num=len(G0s) & 1,
                    )
                    G0s.append(Gp)
                    c0 += pct
            make_identity(nc, ident[:])
            # split preamble loads so later gathers' idx arrive in time
            for b in range(B):
                lo = b * Tb * 8
                nc.sync.dma_start(out=it[:, lo:(b + 1) * Tb * 8],
                                  in_=idxs[:, lo:(b + 1) * Tb * 8])
                nc.sync.dma_start(out=wt[:, 4 * b * Tb:4 * (b + 1) * Tb],
                                  in_=wts[:, 4 * b * Tb:4 * (b + 1) * Tb])

            # chunk sizes: small first chunk (quick pipeline fill), small
            # final chunks (short drain), CT in the middle
            def chunk_sizes(nt, first, last):
                szs = []
                rem = nt
                if first and rem > CT:
                    szs.append(2)
                    rem -= 2
                tail = [2, 1] if (last and rem > 6) else []
                body = rem - sum(tail)
                while body > 0:
                    c = min(CT, body)
                    szs.append(c)
                    body -= c
                szs.extend(tail)
                return szs

            # chunk descriptors: (band, first tile, size)
            chunks = []
            for b in range(B):
                t0 = 0
                for ct in chunk_sizes(Tb, b == 0, b == B - 1):
                    chunks.append((b, b * Tb + t0, ct))
                    t0 += ct
            qn = 0
            for i, (b, tg, ct) in enumerate(chunks):
                src_ap = bass.AP(pm[:].tensor, FOFF[b] * 256,
                                 [[256, NPIX[b] - 1], [1, 512]])
                is_pfx = PFXT and i < 2
                if is_pfx:
                    G = G0s[i]      # gathered up-front via iota idx
                else:
                    G = gpool.tile([P, CT * 512], BF16, tag="G")
                    nc.gpsimd.dma_gather(
                        out_ap=G[:, :ct * 512].rearrange(
                            "p (n e) -> p n e", e=512),
                        in_ap=src_ap,
                        idxs_ap=it[:, tg * 8:(tg + ct) * 8],
                        num_idxs=ct * P,
                        num_idxs_reg=ct * P,
                        elem_size=512,
                        elem_step=256,
                        queue_num=qn,
                    )
                qn ^= 1
                ob = opool.tile([P, CT * P], BF16, tag="ob")
                for g0 in range(0, ct, 4):
                    gw = min(4, ct - g0)
                    ps = ppool.tile([P, 4 * P], F32, tag="ps")
                    for jj in range(gw):
                        j = g0 + jj
                        t = tg + j
                        D = dpool.tile([P, 512], BF16, tag="D")
                        for k in range(4):
                            nc.vector.tensor_scalar_mul(
                                D[:, k * P:(k + 1) * P], ident[:],
                                wt[:, 4 * t + k:4 * t + k + 1])
                        for k in range(4):
                            nc.tensor.matmul(
                                ps[:, jj * P:(jj + 1) * P],
                                G[:, j * 512 + k * P:j * 512 + (k + 1) * P],
                                D[:, k * P:(k + 1) * P],
                                start=(k == 0), stop=(k == 3))
                    nc.scalar.copy(ob[:, g0 * P:(g0 + gw) * P],
                                   ps[:, :gw * P])
                nc.sync.dma_start(
                    out=out[:, tg * P:(tg + ct) * P],
                    in_=ob[:, :ct * P])

    nc.compile()
    return nc


_PROGRAM_CACHE = {}


def _get_program(B, SPANS, Tb):
    key = (B, tuple(SPANS), Tb)
    if key not in _PROGRAM_CACHE:
        _PROGRAM_CACHE[key] = build_program(B, SPANS, Tb)
    return _PROGRAM_CACHE[key]


def kernel(feature_map, target_uv, downscale):
    fm = np.asarray(feature_map, dtype=np.float32)
    uv = np.asarray(target_uv, dtype=np.float32)
    ds = np.float32(np.asarray(downscale).item() if hasattr(downscale, "item")
                    else downscale)
    Cc, H, W = fm.shape
    N = uv.shape[0]
    assert Cc == C and W == W2

    u = (uv[:, 0] / ds).astype(np.float32)
    v = (uv[:, 1] / ds).astype(np.float32)
    ulo = u.astype(np.int32)
    vlo = v.astype(np.int32)
    du = u - ulo.astype(np.float32)
    dv = v - vlo.astype(np.float32)
    # corner order k: 0=(lo,u) 1=(hi,u) 2=(lo,u+1) 3=(hi,u+1)
    w4 = np.stack([(1 - dv) * (1 - du), dv * (1 - du),
                   (1 - dv) * du, dv * du], axis=1).astype(np.float32)

    order = np.argsort(v, kind="stable")
    core_bounds = [(N * k) // N_CORES for k in range(N_CORES + 1)]
    max_core_n = max(core_bounds[k + 1] - core_bounds[k]
                     for k in range(N_CORES))

    # choose band count B so that every band spans <= SPAN_MAX map rows
    B = 2
    while True:
        Tb = max(int(np.ceil(np.ceil(max_core_n / B) / P)), 1)
        NB = Tb * P
        bases = np.zeros((N_CORES, B), dtype=np.int64)
        spans = []
        band_pts = {}
        band_nreal = {}
        ok = True
        for k in range(N_CORES):
            ids = order[core_bounds[k]:core_bounds[k + 1]]
            nb_bounds = [(len(ids) * b) // B for b in range(B + 1)]
            for b in range(B):
                bids = ids[nb_bounds[b]:nb_bounds[b + 1]]
                if len(bids) == 0:
                    bids = ids[:1] if len(ids) else np.array([0], np.int64)
                vb = vlo[bids]
                base = int(vb.min())
                span = int(vb.max()) + 1 - base
                if span > SPAN_MAX:
                    ok = False
                    break
                bases[k, b] = base
                spans.append(span)
                band_nreal[(k, b)] = nb_bounds[b + 1] - nb_bounds[b]
                pad = NB - len(bids)
                band_pts[(k, b)] = np.concatenate(
                    [bids, np.repeat(bids[:1], pad)]) if pad else bids
            if not ok:
                break
        if ok:
            break
        B += 1

    spans2 = np.array(spans, dtype=np.int64).reshape(N_CORES, B)
    SPANS = tuple(int(spans2[:, b].max()) for b in range(B))
    NPIX = [s * W2 for s in SPANS]
    CT = 8
    PFXT = (2 + CT) if Tb > 2 * CT else 0
    PFX = PFXT * 256
    FOFF = [PFX]
    for n in NPIX:
        FOFF.append(FOFF[-1] + n)
    # clamp bases so base + SPANS[b] <= H - 1 (pm pair r uses rows r, r+1)
    for k in range(N_CORES):
        for b in range(B):
            bases[k, b] = min(bases[k, b], H - 1 - SPANS[b])
    Tc = B * Tb

    fmT16 = fm.transpose(1, 2, 0).astype(BFNP)      # [H, W, C] bf16

    in_maps = []
    for k in range(N_CORES):
        pm_k = np.empty((FOFF[-1], 256), dtype=BFNP)
        wts_k = np.empty((P, 4 * Tc), dtype=np.float32)
        idx_k = np.empty((16, Tc * 8), dtype=np.int16)
        for b in range(B):
            base = int(bases[k, b])
            span = SPANS[b]
            blk = np.stack([fmT16[base:base + span],
                            fmT16[base + 1:base + 1 + span]], axis=2)
            bandflat = blk.reshape(span * W2, 256)
            pm_k[FOFF[b]:FOFF[b + 1]] = bandflat
            pts = band_pts[(k, b)]
            if b == 0 and PFXT:
                # duplicate first PFXT*128 points' windows into the prefix:
                # window i at pixel-pair rows [2i, 2i+1]
                p0 = pts[:PFXT * P]
                pix0 = ((vlo[p0] - base).astype(np.int64) * W2 + ulo[p0])
                rows = np.stack([pix0, pix0 + 1], axis=1).reshape(-1)
                pm_k[0:PFX] = bandflat[rows]
            # weights: [p, 4*t] layout (tile-major cols, 4 per tile)
            wb = w4[pts].reshape(Tb, P, 4).transpose(1, 0, 2).reshape(P, Tb * 4)
            wts_k[:, 4 * b * Tb:4 * (b + 1) * Tb] = wb
            # gather indices: element (t*128+p) at [p%16, t*8 + p//16]
            pix = ((vlo[pts] - base).astype(np.int64) * W2
                   + ulo[pts]).astype(np.int16)
            ib = pix.reshape(Tb, 8, 16).transpose(2, 0, 1).reshape(16, Tb * 8)
            idx_k[:, b * Tb * 8:(b + 1) * Tb * 8] = ib
        in_maps.append({"pm": pm_k, "wts": wts_k,
                        "idxs": np.tile(idx_k, (8, 1))})

    nc = _get_program(B, SPANS, Tb)
    res = run_bass_kernel_spmd(nc, in_maps, list(range(N_CORES)))

    out_full = np.empty((C, N), dtype=np.float32)
    for k in range(N_CORES):
        ok_arr = np.asarray(res.results[k]["out"]).astype(np.float32)
        for b in range(B):
            pts = band_pts[(k, b)]
            nreal = band_nreal[(k, b)]
            out_full[:, pts[:nreal]] = ok_arr[:, b * Tb * P:b * Tb * P + nreal]
    return out_full


# revision 11
# speedup vs baseline: 1.0149x; 1.0009x over previous
"""Bilinear resampling kernel for Trainium2 (8 NeuronCores, SPMD).

reference semantics:
    u = target_uv[:, 0] / downscale ; v = target_uv[:, 1] / downscale
    out[c, i] = bilinear sample of feature_map[c] at (v[i], u[i])   -> [C, N]

Strategy (v2 — host-side transpose, bf16, PE blend)
---------------------------------------------------
Host: sort points by v, split into 8 equal per-core chunks, then split each
core's chunk into B bands (span <= 26 map rows so pixel indices fit int16).
For each band ship a PAIR-INTERLEAVED pixel-major bf16 slice pm:
    pm[r*1248 + u] = [fm[:, base+r, u], fm[:, base+r+1, u]]   (256 bf16)
so ONE gather window of 512 bf16 (= 2 consecutive pixel-pairs, 1KB) holds
all four bilinear corners of a point.  Weights (f32) and gather indices
(int16, wrapped [16, n/16] + replicated x8) are precomputed on host.

Device (same program on all 8 cores):
  per chunk of CT tiles (128 points each): one dma_gather pulls 1KB per
  point -> G[p, j, 512] (partition = point).  Blend + transpose happen ON
  THE PE: for each corner k, build D_k = diag(w_k) with one tensor_scalar
  (identity x per-partition weight, 4x DVE mode), then accumulate
      psum[c, p] += G_k[p, c]^T @ D_k        (4 matmuls, one PSUM group)
  Four tiles share one PSUM bank; one ScalarE copy downcasts to bf16 into
  the output buffer, DMA to out.  Output returned bf16, host converts.

Engine budget per core (cost model): DMA ~94us (gather 25.6MB + out 6.4MB),
DVE ~74us, PE ~45us, Act ~30us, Pool ~35us  ->  DMA-bound.
"""

import numpy as np
import ml_dtypes

import concourse.bacc as bacc
import concourse.bass as bass
import concourse.mybir as mybir
import concourse.tile as tile
from concourse.bass_utils import run_bass_kernel_spmd
from concourse.masks import make_identity

C = 128
P = 128
W2 = 1248          # row pitch in pixels
H_FULL = 376
N_CORES = 8
F32 = mybir.dt.float32
BF16 = mybir.dt.bfloat16
I16 = mybir.dt.int16
BFNP = ml_dtypes.bfloat16
SPAN_MAX = 26      # (span-1)*1248 + 1246 <= 32767 (int16 gather idx)


def build_program(B, SPANS, Tb, num_devices=N_CORES, CT=8,
                  gbufs=6, dbufs=12, obufs=6, pbufs=8):
    """SPMD Bass program. SPANS = per-band pm row counts (shared across cores)."""
    if isinstance(SPANS, int):
        SPANS = (SPANS,) * B
    SPANS = tuple(SPANS)
    assert len(SPANS) == B
    PFXT = (2 + CT) if Tb > 2 * CT else 0   # prefix tiles w/ iota-gen indices
    PFX = PFXT * 256                        # prefix pixel-pair rows
    NPIX = [s * W2 for s in SPANS]          # pixel-pairs per band
    FOFF = [PFX]
    for n in NPIX:
        FOFF.append(FOFF[-1] + n)
    Tc = B * Tb

    nc = bacc.Bacc("TRN2", target_bir_lowering=False, debug=False,
                   num_devices=num_devices, num_swdge_queues=2)

    pm = nc.dram_tensor("pm", [FOFF[-1], 256], BF16, kind="ExternalInput")
    wts = nc.dram_tensor("wts", [P, 4 * Tc], F32, kind="ExternalInput")
    idxs = nc.dram_tensor("idxs", [P, Tc * 8], I16, kind="ExternalInput")
    out = nc.dram_tensor("out", [P, Tc * P], BF16, kind="ExternalOutput")

    with tile.TileContext(nc) as tc:
        with (
            tc.tile_pool(name="const", bufs=1) as cpool,
            tc.tile_pool(name="gather", bufs=gbufs) as gpool,
            tc.tile_pool(name="diag", bufs=dbufs) as dpool,
            tc.tile_pool(name="obuf", bufs=obufs) as opool,
            tc.tile_pool(name="psum", bufs=pbufs, space="PSUM") as ppool,
        ):
            ident = cpool.tile([P, P], BF16, tag="ident")
            wt = cpool.tile([P, 4 * Tc], F32, tag="wt")
            it = cpool.tile([P, Tc * 8], I16, tag="it")
            G0s = None
            if PFXT:
                # iota-generated idx for the prefix chunk: no DMA dependency.
                # replicated wrapped layout: it0[p, c] = 32*c + 2*(p % 16)
                it0 = cpool.tile([P, PFXT * 8], I16, tag="it0")
                itp = cpool.tile([P, PFXT * 8], I16, tag="itp")
                nc.gpsimd.iota(itp[:], pattern=[[0, PFXT * 8]],
                               base=0, channel_multiplier=1)
                nc.vector.tensor_scalar(itp[:], itp[:], 15, None,
                                        op0=mybir.AluOpType.bitwise_and)
                nc.gpsimd.iota(it0[:], pattern=[[32, PFXT * 8]],
                               base=0, channel_multiplier=0)
                nc.vector.scalar_tensor_tensor(
                    it0[:], itp[:], 2, it0[:],
                    op0=mybir.AluOpType.mult, op1=mybir.AluOpType.add)
                # issue the prefix gathers before anything else queues on
                # Pool; they cover the first chunks (sizes 2 then CT)
                src_pfx = bass.AP(pm[:].tensor, 0,
                                  [[256, PFX - 1], [1, 512]])
                G0s = []
                c0 = 0
                for pct in (2, CT):
                    Gp = gpool.tile([P, CT * 512], BF16, tag="G")
                    nc.gpsimd.dma_gather(
                        out_ap=Gp[:, :pct * 512].rearrange(
                            "p (n e) -> p n e", e=512),
                        in_ap=src_pfx,
                        idxs_ap=it0[:, c0 * 8:(c0 + pct) * 8],
                        num_idxs=pct * P,
                        num_idxs_reg=pct * P,
                        elem_size=512,
                        elem_step=256,
                        queue_num=len(G0s) & 1,
                    )
                    G0s.append(Gp)
                    c0 += pct
            make_identity(nc, ident[:])
            # single whole-tensor preamble loads: the iota prefix hides
            # the fill, so the full idx/wts arrive well before needed
            nc.sync.dma_start(out=it[:], in_=idxs[:])
            nc.sync.dma_start(out=wt[:], in_=wts[:])

            # chunk sizes: small first chunk (quick pipeline fill), small
            # final chunks (short drain), CT in the middle
            def chunk_sizes(nt, first, last):
                szs = []
                rem = nt
                if first and rem > CT:
                    szs.append(2)
                    rem -= 2
                tail = [2, 1] if (last and rem > 6) else []
                body = rem - sum(tail)
                while body > 0:
                    c = min(CT, body)
                    szs.append(c)
                    body -= c
                szs.extend(tail)
                return szs

            # chunk descriptors: (band, first tile, size)
            chunks = []
            for b in range(B):
                t0 = 0
                for ct in chunk_sizes(Tb, b == 0, b == B - 1):
                    chunks.append((b, b * Tb + t0, ct))
                    t0 += ct
            qn = 0
            for i, (b, tg, ct) in enumerate(chunks):
                src_ap = bass.AP(pm[:].tensor, FOFF[b] * 256,
                                 [[256, NPIX[b] - 1], [1, 512]])
                is_pfx = PFXT and i < 2
                if is_pfx:
                    G = G0s[i]      # gathered up-front via iota idx
                else:
                    G = gpool.tile([P, CT * 512], BF16, tag="G")
                    nc.gpsimd.dma_gather(
                        out_ap=G[:, :ct * 512].rearrange(
                            "p (n e) -> p n e", e=512),
                        in_ap=src_ap,
                        idxs_ap=it[:, tg * 8:(tg + ct) * 8],
                        num_idxs=ct * P,
                        num_idxs_reg=ct * P,
                        elem_size=512,
                        elem_step=256,
                        queue_num=qn,
                    )
                qn ^= 1
                ob = opool.tile([P, CT * P], BF16, tag="ob")
                for g0 in range(0, ct, 4):
                    gw = min(4, ct - g0)
                    ps = ppool.tile([P, 4 * P], F32, tag="ps")
                    for jj in range(gw):
                        j = g0 + jj
                        t = tg + j
                        D = dpool.tile([P, 512], BF16, tag="D")
                        for k in range(4):
                            nc.vector.tensor_scalar_mul(
                                D[:, k * P:(k + 1) * P], ident[:],
                                wt[:, 4 * t + k:4 * t + k + 1])
                        for k in range(4):
                            nc.tensor.matmul(
                                ps[:, jj * P:(jj + 1) * P],
                                G[:, j * 512 + k * P:j * 512 + (k + 1) * P],
                                D[:, k * P:(k + 1) * P],
                                start=(k == 0), stop=(k == 3))
                    nc.scalar.copy(ob[:, g0 * P:(g0 + gw) * P],
                                   ps[:, :gw * P])
                nc.sync.dma_start(
                    out=out[:, tg * P:(tg + ct) * P],
                    in_=ob[:, :ct * P])

    nc.compile()
    return nc


_PROGRAM_CACHE = {}


def _get_program(B, SPANS, Tb):
    key = (B, tuple(SPANS), Tb)
    if key not in _PROGRAM_CACHE:
        _PROGRAM_CACHE[key] = build_program(B, SPANS, Tb)
    return _PROGRAM_CACHE[key]


def kernel(feature_map, target_uv, downscale):
    fm = np.asarray(feature_map, dtype=np.float32)
    uv = np.asarray(target_uv, dtype=np.float32)
    ds = np.float32(np.asarray(downscale).item() if hasattr(downscale, "item")
                    else downscale)
    Cc, H, W = fm.shape
    N = uv.shape[0]
    assert Cc == C and W == W2

    u = (uv[:, 0] / ds).astype(np.float32)
    v = (uv[:, 1] / ds).astype(np.float32)
    ulo = u.astype(np.int32)
    vlo = v.astype(np.int32)
    du = u - ulo.astype(np.float32)
    dv = v - vlo.astype(np.float32)
    # corner order k: 0=(lo,u) 1=(hi,u) 2=(lo,u+1) 3=(hi,u+1)
    w4 = np.stack([(1 - dv) * (1 - du), dv * (1 - du),
                   (1 - dv) * du, dv * du], axis=1).astype(np.float32)

    order = np.argsort(v, kind="stable")
    core_bounds = [(N * k) // N_CORES for k in range(N_CORES + 1)]
    max_core_n = max(core_bounds[k + 1] - core_bounds[k]
                     for k in range(N_CORES))

    # choose band count B so that every band spans <= SPAN_MAX map rows
    B = 2
    while True:
        Tb = max(int(np.ceil(np.ceil(max_core_n / B) / P)), 1)
        NB = Tb * P
        bases = np.zeros((N_CORES, B), dtype=np.int64)
        spans = []
        band_pts = {}
        band_nreal = {}
        ok = True
        for k in range(N_CORES):
            ids = order[core_bounds[k]:core_bounds[k + 1]]
            nb_bounds = [(len(ids) * b) // B for b in range(B + 1)]
            for b in range(B):
                bids = ids[nb_bounds[b]:nb_bounds[b + 1]]
                if len(bids) == 0:
                    bids = ids[:1] if len(ids) else np.array([0], np.int64)
                vb = vlo[bids]
                base = int(vb.min())
                span = int(vb.max()) + 1 - base
                if span > SPAN_MAX:
                    ok = False
                    break
                bases[k, b] = base
                spans.append(span)
                band_nreal[(k, b)] = nb_bounds[b + 1] - nb_bounds[b]
                pad = NB - len(bids)
                band_pts[(k, b)] = np.concatenate(
                    [bids, np.repeat(bids[:1], pad)]) if pad else bids
            if not ok:
                break
        if ok:
            break
        B += 1

    spans2 = np.array(spans, dtype=np.int64).reshape(N_CORES, B)
    SPANS = tuple(int(spans2[:, b].max()) for b in range(B))
    NPIX = [s * W2 for s in SPANS]
    CT = 8
    PFXT = (2 + CT) if Tb > 2 * CT else 0
    PFX = PFXT * 256
    FOFF = [PFX]
    for n in NPIX:
        FOFF.append(FOFF[-1] + n)
    # clamp bases so base + SPANS[b] <= H - 1 (pm pair r uses rows r, r+1)
    for k in range(N_CORES):
        for b in range(B):
            bases[k, b] = min(bases[k, b], H - 1 - SPANS[b])
    Tc = B * Tb

    fmT16 = fm.transpose(1, 2, 0).astype(BFNP)      # [H, W, C] bf16

    in_maps = []
    for k in range(N_CORES):
        pm_k = np.empty((FOFF[-1], 256), dtype=BFNP)
        wts_k = np.empty((P, 4 * Tc), dtype=np.float32)
        idx_k = np.empty((16, Tc * 8), dtype=np.int16)
        for b in range(B):
            base = int(bases[k, b])
            span = SPANS[b]
            blk = np.stack([fmT16[base:base + span],
                            fmT16[base + 1:base + 1 + span]], axis=2)
            bandflat = blk.reshape(span * W2, 256)
            pm_k[FOFF[b]:FOFF[b + 1]] = bandflat
            pts = band_pts[(k, b)]
            if b == 0 and PFXT:
                # duplicate first PFXT*128 points' windows into the prefix:
                # window i at pixel-pair rows [2i, 2i+1]
                p0 = pts[:PFXT * P]
                pix0 = ((vlo[p0] - base).astype(np.int64) * W2 + ulo[p0])
                rows = np.stack([pix0, pix0 + 1], axis=1).reshape(-1)
                pm_k[0:PFX] = bandflat[rows]
            # weights: [p, 4*t] layout (tile-major cols, 4 per tile)
            wb = w4[pts].reshape(Tb, P, 4).transpose(1, 0, 2).reshape(P, Tb * 4)
            wts_k[:, 4 * b * Tb:4 * (b + 1) * Tb] = wb
            # gather indices: element (t*128+p) at [p%16, t*8 + p//16]
            pix = ((vlo[pts] - base).astype(np.int64) * W2
                   + ulo[pts]).astype(np.int16)
            ib = pix.reshape(Tb, 8, 16).transpose(2, 0, 1).reshape(16, Tb * 8)
            idx_k[:, b * Tb * 8:(b + 1) * Tb * 8] = ib
        in_maps.append({"pm": pm_k, "wts": wts_k,
                        "idxs": np.tile(idx_k, (8, 1))})

    nc = _get_program(B, SPANS, Tb)
    res = run_bass_kernel_spmd(nc, in_maps, list(range(N_CORES)))

    out_full = np.empty((C, N), dtype=np.float32)
    for k in range(N_CORES):
        ok_arr = np.asarray(res.results[k]["out"]).astype(np.float32)
        for b in range(B):
            pts = band_pts[(k, b)]
            nreal = band_nreal[(k, b)]
            out_full[:, pts[:nreal]] = ok_arr[:, b * Tb * P:b * Tb * P + nreal]
    return out_full
